# revision 5
# baseline (speedup 1.0000x reference)
"""Trainium2 Bass kernel for nn_ChADALINE.

Reference computes, for x:[B,1,IN], weight/bias:[IN,OUT]:
    z[b,o,i] = x[b,0,i] * weight[i,o] + bias[i,o]
    chi[b,o] = Choquet integral of z[b,o,:] with cardinality measure
    out      = sigmoid(chi)

The Choquet integral with mu(A_i) = (n-i+1)/n telescopes to the plain mean:
    sum_i (z_(i) - z_(i-1)) * (n-i+1)/n = (1/n) * sum_i z_(i) = mean(z)
and the sum of sorted values equals the unsorted sum, so the sort drops out:
    out = sigmoid((x @ weight + bias.sum(axis=0)) / IN)        # [B, OUT]

Device strategy: shard the OUT dimension over the 8 cores (weight/bias column
slices per core, x replicated).  Each core computes
    out_c[o, b] = sigmoid((W_c^T x^T + colsum(bias_c)) / IN)
with one PSUM accumulation over 8 K-tiles on the PE, the bias column-sum
folded in via a ones-vector matmul, and the final sigmoid fused on the
scalar engine (activation with per-partition bias + scale).  Inputs are
fed to the device as bf16 (error ~5e-6 on an output of magnitude ~0.5).
"""

import numpy as np
import ml_dtypes

import concourse.bass as bass
import concourse.mybir as mybir
import concourse.tile as tile
from concourse import bacc
from concourse.bass_utils import run_bass_kernel_spmd

B, IN, OUT = 256, 1024, 1024
NCORES = 8
OSL = OUT // NCORES  # 128 output columns per core
P = 128              # partition count
KT = IN // P         # 8 contraction tiles

_CACHE: dict = {}


def _build_nc() -> bass.Bass:
    nc = bacc.Bacc(trn_type="TRN2", target_bir_lowering=False, debug=False)

    # Packed DRAM layouts (host pre-packs):
    #   xt[j, k*B + b]   = x[b, k*P + j]     (x transposed, K-tiles side by side)
    #   w [j, k*OSL + o] = weight[k*P + j, c*OSL + o]
    #   bs[j, k*OSL + o] = bias  [k*P + j, c*OSL + o]
    xt = nc.dram_tensor("xt", [P, KT * B], mybir.dt.bfloat16, kind="ExternalInput")
    w = nc.dram_tensor("w", [P, KT * OSL], mybir.dt.bfloat16, kind="ExternalInput")
    bs = nc.dram_tensor("bs", [P, KT * OSL], mybir.dt.bfloat16, kind="ExternalInput")
    out = nc.dram_tensor("out", [OSL, B], mybir.dt.float32, kind="ExternalOutput")

    with tile.TileContext(nc) as tc:
        with (
            tc.tile_pool(name="sb", bufs=1) as pool,
            tc.tile_pool(name="ps", bufs=1, space="PSUM") as psum,
        ):
            xt_sb = pool.tile([P, KT * B], mybir.dt.bfloat16)
            w_sb = pool.tile([P, KT * OSL], mybir.dt.bfloat16)
            b_sb = pool.tile([P, KT * OSL], mybir.dt.bfloat16)
            ones = pool.tile([P, B], mybir.dt.bfloat16)

            nc.sync.dma_start(xt_sb[:], xt.ap())
            nc.sync.dma_start(w_sb[:], w.ap())
            nc.sync.dma_start(b_sb[:], bs.ap())
            nc.vector.memset(ones[:], 1.0)

            psum_main = psum.tile([P, B], mybir.dt.float32)

            # out[o, b] = sum_k (w_tile_k)^T @ xt_tile_k
            #           + sum_k (bs_tile_k)^T @ ones        (= colsum(bias_c)[o])
            for k in range(KT):
                nc.tensor.matmul(
                    psum_main[:],
                    w_sb[:, k * OSL : (k + 1) * OSL],
                    xt_sb[:, k * B : (k + 1) * B],
                    start=(k == 0),
                    stop=False,
                )
            for k in range(KT):
                nc.tensor.matmul(
                    psum_main[:],
                    b_sb[:, k * OSL : (k + 1) * OSL],
                    ones[:],
                    start=False,
                    stop=(k == KT - 1),
                )

            out_sb = pool.tile([P, B], mybir.dt.float32)
            nc.scalar.activation(
                out_sb[:],
                psum_main[:],
                mybir.ActivationFunctionType.Sigmoid,
                bias=0.0,
                scale=1.0 / IN,
            )
            nc.sync.dma_start(out.ap(), out_sb[:])

    nc.compile()
    return nc


def _get_nc() -> bass.Bass:
    if "nc" not in _CACHE:
        _CACHE["nc"] = _build_nc()
    return _CACHE["nc"]


def _pack_kmaj(a: np.ndarray) -> np.ndarray:
    """[IN, C] -> [P, KT*C] with layout [j, k*C + c] = a[k*P + j, c], bf16."""
    n, c = a.shape
    kt = n // P
    packed = a.reshape(kt, P, c).transpose(1, 0, 2).reshape(P, kt * c)
    return np.ascontiguousarray(packed.astype(ml_dtypes.bfloat16))


def kernel(x: np.ndarray, weight: np.ndarray, bias: np.ndarray, **run_kwargs):
    x2 = np.asarray(x).reshape(B, IN)
    weight = np.asarray(weight)
    bias = np.asarray(bias)

    xt_packed = _pack_kmaj(x2.T)  # [P, KT*B], shared by all cores
    in_maps = []
    for c in range(NCORES):
        sl = slice(c * OSL, (c + 1) * OSL)
        in_maps.append(
            {
                "xt": xt_packed,
                "w": _pack_kmaj(weight[:, sl]),
                "bs": _pack_kmaj(bias[:, sl]),
            }
        )

    nc = _get_nc()
    res = run_bass_kernel_spmd(nc, in_maps, core_ids=list(range(NCORES)), **run_kwargs)
    out = np.empty((B, OUT), dtype=np.float32)
    for c in range(NCORES):
        out[:, c * OSL : (c + 1) * OSL] = res.results[c]["out"].T
    if run_kwargs:
        return out, res
    return out


# revision 6
# speedup vs baseline: 1.0247x; 1.0247x over previous
"""Trainium2 Bass kernel for nn_ChADALINE.

Reference computes, for x:[B,1,IN], weight/bias:[IN,OUT]:
    z[b,o,i] = x[b,0,i] * weight[i,o] + bias[i,o]
    chi[b,o] = Choquet integral of z[b,o,:] with cardinality measure
    out      = sigmoid(chi)

The Choquet integral with mu(A_i) = (n-i+1)/n telescopes to the plain mean:
    sum_i (z_(i) - z_(i-1)) * (n-i+1)/n = (1/n) * sum_i z_(i) = mean(z)
and the sum of sorted values equals the unsorted sum, so the sort drops out:
    out = sigmoid((x @ weight + bias.sum(axis=0)) / IN)        # [B, OUT]

Device strategy: shard the OUT dimension over the 8 cores (weight/bias column
slices per core, x replicated).  Each core computes
    out_c[o, b] = sigmoid((W_c^T x^T + colsum(bias_c)) / IN)
with one PSUM accumulation over 8 K-tiles on the PE, the bias column-sum
folded in via a ones-vector matmul, and the final sigmoid fused on the
scalar engine (activation with per-partition bias + scale).  Inputs are
fed to the device as bf16 (error ~5e-6 on an output of magnitude ~0.5).
"""

import numpy as np
import ml_dtypes

import concourse.bass as bass
import concourse.mybir as mybir
import concourse.tile as tile
from concourse import bacc
from concourse.bass_utils import run_bass_kernel_spmd

B, IN, OUT = 256, 1024, 1024
NCORES = 8
OSL = OUT // NCORES  # 128 output columns per core
P = 128              # partition count
KT = IN // P         # 8 contraction tiles

_CACHE: dict = {}


def _build_nc() -> bass.Bass:
    nc = bacc.Bacc(
        trn_type="TRN2", target_bir_lowering=False, debug=False, enable_asserts=False
    )

    # Packed DRAM layouts (host pre-packs):
    #   xt[j, k*B + b]   = x[b, k*P + j]     (x transposed, K-tiles side by side)
    #   w [j, k*OSL + o] = weight[k*P + j, c*OSL + o]
    #   bs[j, k*OSL + o] = bias  [k*P + j, c*OSL + o]
    xt = nc.dram_tensor("xt", [P, KT * B], mybir.dt.bfloat16, kind="ExternalInput")
    w = nc.dram_tensor("w", [P, KT * OSL], mybir.dt.bfloat16, kind="ExternalInput")
    bs = nc.dram_tensor("bs", [P, KT * OSL], mybir.dt.bfloat16, kind="ExternalInput")
    out = nc.dram_tensor("out", [OSL, B], mybir.dt.float32, kind="ExternalOutput")

    with tile.TileContext(nc) as tc:
        with (
            tc.tile_pool(name="sb", bufs=1) as pool,
            tc.tile_pool(name="ps", bufs=1, space="PSUM") as psum,
        ):
            xt_sb = pool.tile([P, KT * B], mybir.dt.bfloat16)
            w_sb = pool.tile([P, KT * OSL], mybir.dt.bfloat16)
            b_sb = pool.tile([P, KT * OSL], mybir.dt.bfloat16)
            ones = pool.tile([P, B], mybir.dt.bfloat16)

            nc.sync.dma_start(xt_sb[:], xt.ap())
            nc.sync.dma_start(w_sb[:], w.ap())
            nc.sync.dma_start(b_sb[:], bs.ap())
            nc.vector.memset(ones[:], 1.0)

            psum_main = psum.tile([P, B], mybir.dt.float32)

            # out[o, b] = sum_k (w_tile_k)^T @ xt_tile_k
            #           + sum_k (bs_tile_k)^T @ ones        (= colsum(bias_c)[o])
            for k in range(KT):
                nc.tensor.matmul(
                    psum_main[:],
                    w_sb[:, k * OSL : (k + 1) * OSL],
                    xt_sb[:, k * B : (k + 1) * B],
                    start=(k == 0),
                    stop=False,
                )
            for k in range(KT):
                nc.tensor.matmul(
                    psum_main[:],
                    b_sb[:, k * OSL : (k + 1) * OSL],
                    ones[:],
                    start=False,
                    stop=(k == KT - 1),
                )

            out_sb = pool.tile([P, B], mybir.dt.float32)
            nc.scalar.activation(
                out_sb[:],
                psum_main[:],
                mybir.ActivationFunctionType.Sigmoid,
                bias=0.0,
                scale=1.0 / IN,
            )
            nc.sync.dma_start(out.ap(), out_sb[:])

    nc.compile()
    return nc


def _get_nc() -> bass.Bass:
    if "nc" not in _CACHE:
        _CACHE["nc"] = _build_nc()
    return _CACHE["nc"]


def _pack_kmaj(a: np.ndarray) -> np.ndarray:
    """[IN, C] -> [P, KT*C] with layout [j, k*C + c] = a[k*P + j, c], bf16."""
    n, c = a.shape
    kt = n // P
    packed = a.reshape(kt, P, c).transpose(1, 0, 2).reshape(P, kt * c)
    return np.ascontiguousarray(packed.astype(ml_dtypes.bfloat16))


def kernel(x: np.ndarray, weight: np.ndarray, bias: np.ndarray, **run_kwargs):
    x2 = np.asarray(x).reshape(B, IN)
    weight = np.asarray(weight)
    bias = np.asarray(bias)

    xt_packed = _pack_kmaj(x2.T)  # [P, KT*B], shared by all cores
    in_maps = []
    for c in range(NCORES):
        sl = slice(c * OSL, (c + 1) * OSL)
        in_maps.append(
            {
                "xt": xt_packed,
                "w": _pack_kmaj(weight[:, sl]),
                "bs": _pack_kmaj(bias[:, sl]),
            }
        )

    nc = _get_nc()
    res = run_bass_kernel_spmd(nc, in_maps, core_ids=list(range(NCORES)), **run_kwargs)
    out = np.empty((B, OUT), dtype=np.float32)
    for c in range(NCORES):
        out[:, c * OSL : (c + 1) * OSL] = res.results[c]["out"].T
    if run_kwargs:
        return out, res
    return out
